# revision 9
# baseline (speedup 1.0000x reference)
"""Trainium2 Bass kernel: multi-head attention with Toeplitz relative bias.

Problem: B=16, L=1024, F=512, H=8, D=64 ViT patch attention.
Sharding: data-parallel over batch, 2 batches per core across 8 cores.

Device-side design (per core, fully unrolled Tile program):
  - Host pre-transposes inputs to xT [F, L] (bf16); projections contract F on
    SBUF partitions.
  - qT/kT computed transposed ([fout, L]): head pair stacked on partitions
    (64 rows each); scores use K=64 matmuls with matching partition bases, so
    no zero padding or memset is needed.
  - Scores computed transposed [k, q] (k on partitions) so attn@v needs no
    transpose of the attention matrix. ACT does exp (its only job); DVE
    multiplies in the host-precomputed exp(bias) (fp16, 2x DVE mode).
  - attn@v computed TRANSPOSED: stationary = [v | ones] (ones columns are
    free: matmul cost depends only on moving cols), moving = exp tiles
    [k, q].  Output psum [128, q]: rows 0:64 = x^T (unnormalized), rows
    64:128 = softmax denominator replicated.  Normalize = DVE reciprocal
    (psum->sbuf, same base) + DVE multiply (psum in0 + sbuf in1, mixed-space
    cross-base, verified legal).  Result lands directly in xatT layout for
    the output projection -- the PE transpose phase of the previous design is
    gone.
  - Head loop is software-pipelined: window (h,b) emits scores(h,b) while
    attn@v of the previous window's head drains, so ACT (the bottleneck at
    ~1us per exp tile) never starves.
  - No max-subtraction in softmax: |scores| <~ 1.5 by construction.
"""

import sys

for _p in ("/opt/trn_rl_repo",):
    if _p not in sys.path:
        sys.path.insert(0, _p)

import numpy as np
import ml_dtypes

import concourse.bass as bass
import concourse.mybir as mybir
import concourse.tile as tile
from concourse import bacc
from concourse.bass_utils import run_bass_kernel_spmd

B, L, F, H, D = 16, 1024, 512, 8, 64
NX, NY = 32, 32
NCORES = 8
BPC = B // NCORES  # batches per core
FP32 = mybir.dt.float32
BF16 = mybir.dt.bfloat16
FP16 = mybir.dt.float16
Exp = mybir.ActivationFunctionType.Exp
Add = mybir.AluOpType.add
Mult = mybir.AluOpType.mult
Bypass = mybir.AluOpType.bypass


def _build():
    nc = bacc.Bacc("TRN2", target_bir_lowering=False, debug=False)

    xqT_d = nc.dram_tensor("xqT", [BPC, F, L], BF16, kind="ExternalInput").ap()
    xkvT_d = nc.dram_tensor("xkvT", [BPC, F, L], BF16, kind="ExternalInput").ap()
    Wq_d = nc.dram_tensor("Wq", [F, F], BF16, kind="ExternalInput").ap()
    Wk_d = nc.dram_tensor("Wk", [F, F], BF16, kind="ExternalInput").ap()
    Wv_d = nc.dram_tensor("Wv", [F, F], BF16, kind="ExternalInput").ap()
    Wo_d = nc.dram_tensor("Wo", [F, F], BF16, kind="ExternalInput").ap()
    bq_d = nc.dram_tensor("bq", [F], FP32, kind="ExternalInput").ap()
    bk_d = nc.dram_tensor("bk", [F], FP32, kind="ExternalInput").ap()
    bvb_d = nc.dram_tensor("bvb", [128, 2 * F], BF16, kind="ExternalInput").ap()
    bob_d = nc.dram_tensor("bob", [128, 2 * F], BF16, kind="ExternalInput").ap()
    biasT_d = nc.dram_tensor("biasT", [H, L, L], FP16, kind="ExternalInput").ap()
    out_d = nc.dram_tensor("out", [BPC, L, F], BF16, kind="ExternalOutput").ap()

    with tile.TileContext(nc) as tc:
        with (
            tc.tile_pool(name="const", bufs=1) as cpool,
            tc.tile_pool(name="xin", bufs=2) as xpool,
            tc.tile_pool(name="qkv", bufs=2) as qpool,
            tc.tile_pool(name="bias", bufs=4) as bpool,
            tc.tile_pool(name="es", bufs=3) as espool,
            tc.tile_pool(name="exp", bufs=12) as epool,
            tc.tile_pool(name="rc", bufs=2) as rcpool,
            tc.tile_pool(name="os", bufs=2) as ospool,
            tc.tile_pool(name="psS", bufs=2, space="PSUM") as psS,
            tc.tile_pool(name="psU", bufs=2, space="PSUM") as psU,
        ):
            # ---- constant loads (DMA) ----
            Wq_s = cpool.tile([128, 4 * F], BF16, tag="Wq")
            Wk_s = cpool.tile([128, 4 * F], BF16, tag="Wk")
            Wv_s = cpool.tile([128, 4 * F], BF16, tag="Wv")
            Wo_s = cpool.tile([128, 4 * F], BF16, tag="Wo")

            def load_w(w_s, w_d):
                nc.sync.dma_start(
                    out=w_s[:].rearrange("p (c n) -> p c n", c=4),
                    in_=w_d.rearrange("(c p) n -> p c n", c=4),
                )

            # bias for head 0 staged first: needed ~9us in
            bias_tiles = {}

            def stage_bias(h):
                tiles = []
                for hh in range(2):
                    bt = bpool.tile([128, 4 * L], FP16, tag="bias")
                    nc.sync.dma_start(
                        out=bt[:].rearrange("p (t q) -> p t q", t=4),
                        in_=biasT_d[h, hh * 512 : (hh + 1) * 512, :].rearrange(
                            "(t p) q -> p t q", t=4
                        ),
                    )
                    tiles.append(bt)
                bias_tiles[h] = tiles

            stage_bias(0)
            load_w(Wq_s, Wq_d)
            load_w(Wk_s, Wk_d)
            bq_s = cpool.tile([128, 4], FP32, tag="bq")
            bk_s = cpool.tile([128, 4], FP32, tag="bk")
            for b_s, b_d in ((bq_s, bq_d), (bk_s, bk_d)):
                nc.sync.dma_start(out=b_s[:], in_=b_d.rearrange("(c p) -> p c", p=128))

            xq, xkv = [], []
            for b in range(BPC):
                xq_t = xpool.tile([128, 4 * L], BF16, tag="xq")
                xkv_t = xpool.tile([128, 4 * L], BF16, tag="xkv")
                nc.sync.dma_start(
                    out=xkv_t[:].rearrange("p (c l) -> p c l", c=4),
                    in_=xkvT_d[b].rearrange("(c p) l -> p c l", c=4),
                )
                nc.sync.dma_start(
                    out=xq_t[:].rearrange("p (c l) -> p c l", c=4),
                    in_=xqT_d[b].rearrange("(c p) l -> p c l", c=4),
                )
                xq.append(xq_t)
                xkv.append(xkv_t)
            load_w(Wv_s, Wv_d)
            bvb_s = cpool.tile([128, 2 * F], BF16, tag="bvb")
            nc.sync.dma_start(out=bvb_s[:], in_=bvb_d)
            stage_bias(1)
            load_w(Wo_s, Wo_d)
            bob_s = cpool.tile([128, 2 * F], BF16, tag="bob")
            nc.sync.dma_start(out=bob_s[:], in_=bob_d)

            # ---- persistent per-batch tiles ----
            qT, kT, vAug, xatT = [], [], [], []
            for b in range(BPC):
                qT_t = qpool.tile([128, 4 * L], BF16, tag="qT")
                kT_t = qpool.tile([128, 4 * L], BF16, tag="kT")
                # vAug [128, lt(8) x h(8) x 128]: cols 0:64 v, 64:128 ones
                vAug_t = qpool.tile([128, 8 * 8 * 128], FP16, tag="vAug")
                xatT_t = qpool.tile([128, 4 * L], BF16, tag="xatT")
                qT.append(qT_t)
                kT.append(kT_t)
                vAug.append(vAug_t)
                xatT.append(xatT_t)

            for b in range(BPC):
                nc.gpsimd.memset(
                    vAug[b][:].rearrange("p (t h c) -> p t h c", t=8, h=8)[
                        :, :, :, 64:128
                    ],
                    1.0,
                )

            # ---- background PE work generators (interleaved into windows) ----
            def v_proj_steps(b):
                # v natural [L, F] (+bv): per lt-pair one psS tile [128, 1024]
                for ltp in range(4):
                    pv = psS.tile([128, 1024], FP32, tag="ps")
                    for half in range(2):
                        lt = 2 * ltp + half
                        for kc in range(4):
                            nc.tensor.matmul(
                                pv[:, half * 512 : (half + 1) * 512],
                                xkv[b][:, kc * L + lt * 128 : kc * L + (lt + 1) * 128],
                                Wv_s[:, kc * F : (kc + 1) * F],
                                start=(kc == 0),
                                stop=(kc == 3),
                            )
                    # evict both halves in one DVE op (+bv broadcast), fp16
                    nc.vector.scalar_tensor_tensor(
                        vAug[b][:, ltp * 2048 : (ltp + 1) * 2048]
                        .rearrange("p (t h c) -> p t h c", t=2, h=8)[:, :, :, 0:64],
                        pv[:],
                        1.0,
                        bvb_s[:],
                        Bypass,
                        Add,
                    )
                    yield

            def qk_proj_steps(fo, b):
                # qT/kT transposed [fout, L]; head pair stacked on partitions
                for which, w_s, b_s, x_t, dst in (
                    ("q", Wq_s, bq_s, xq[b], qT[b]),
                    ("k", Wk_s, bk_s, xkv[b], kT[b]),
                ):
                    pq = psS.tile([128, 1024], FP32, tag="ps")
                    for lc in range(2):
                        for kc in range(4):
                            nc.tensor.matmul(
                                pq[:, lc * 512 : (lc + 1) * 512],
                                w_s[:, kc * F + fo * 128 : kc * F + (fo + 1) * 128],
                                x_t[:, kc * L + lc * 512 : kc * L + (lc + 1) * 512],
                                start=(kc == 0),
                                stop=(kc == 3),
                            )
                        if lc == 0:
                            yield
                    nc.vector.tensor_scalar_add(
                        dst[:, fo * L : (fo + 1) * L], pq[:], b_s[:, fo : fo + 1]
                    )
                    yield

            def phase_c_steps(b):
                # out projection: out[q, f] = sum_c xatT_c^T @ Wo_c (+bo)
                for ltp in range(4):
                    po = psS.tile([128, 1024], FP32, tag="ps")
                    for half in range(2):
                        lt = 2 * ltp + half
                        for c in range(4):
                            nc.tensor.matmul(
                                po[:, half * 512 : (half + 1) * 512],
                                xatT[b][:, c * L + lt * 128 : c * L + (lt + 1) * 128],
                                Wo_s[:, c * F : (c + 1) * F],
                                start=(c == 0),
                                stop=(c == 3),
                            )
                        yield
                    os_t = ospool.tile([128, 1024], BF16, tag="os")
                    nc.vector.scalar_tensor_tensor(
                        os_t[:], po[:], 1.0, bob_s[:], Bypass, Add
                    )
                    nc.sync.dma_start(
                        out=out_d[b, ltp * 256 : (ltp + 1) * 256, :].rearrange(
                            "(t p) f -> p t f", t=2
                        ),
                        in_=os_t[:].rearrange("p (t f) -> p t f", t=2),
                    )
                    yield

            # background queue: list of generators, consumed a few steps/slot
            bg = []

            def bg_step(n=1):
                for _ in range(n):
                    while bg:
                        try:
                            next(bg[0])
                            break
                        except StopIteration:
                            bg.pop(0)
                    else:
                        return

            # ---- window schedule ----
            # window w (w = 0..15): scores/exp for (h, b) = (w//2, w%2),
            # attn@v for the previous window's (h, b).
            # qk fo0/b0 emitted fully up front (window 0 needs it);
            # qk fo0/b1 + v b0 + v b1 drain during window 0 (needed at
            # windows 1, 1, 2 respectively).
            for _ in qk_proj_steps(0, 0):
                pass
            bg.append(qk_proj_steps(0, 1))
            bg.append(v_proj_steps(0))
            bg.append(v_proj_steps(1))
            # remaining projections: fo chunk f first used at window 4f
            pending_proj = {1: (1, 0), 2: (1, 1), 5: (2, 0), 6: (2, 1),
                            9: (3, 0), 10: (3, 1)}

            ex_tiles = {}  # (b, kt) -> ex tile for the in-flight head
            prev = None  # (h, b, psU_tile) of the window being drained

            def emit_attnv_slot(h, b, pu, kt):
                for qc in range(2):
                    nc.tensor.matmul(
                        pu[:, qc * 512 : (qc + 1) * 512],
                        vAug[b][:, (kt * 8 + h) * 128 : (kt * 8 + h + 1) * 128],
                        ex_tiles[(b, kt)][:, qc * 512 : (qc + 1) * 512],
                        start=(kt == 0),
                        stop=(kt == 7),
                    )

            def emit_norm(h, b, pu):
                # psum rows 0:64 = x^T, rows 64:128 = denom (replicated)
                hp = (h % 2) * 64
                c = h // 2
                rc_t = rcpool.tile([128, 1024], FP16, tag="rc")
                with nc.allow_low_precision(
                    reason="softmax recip/normalize; fp16 denom recip adds <6e-4 rel"
                ):
                    nc.vector.reciprocal(rc_t[hp : hp + 64, :], pu[64:128, :])
                    nc.vector.tensor_tensor(
                        xatT[b][hp : hp + 64, c * L : (c + 1) * L],
                        pu[0:64, :],
                        rc_t[hp : hp + 64, :],
                        Mult,
                    )

            for w in range(2 * H):
                h, b = w // 2, w % 2
                hp = (h % 2) * 64
                hc = (h // 2) * L
                if w in pending_proj:
                    fo, pb = pending_proj[w]
                    bg.append(qk_proj_steps(fo, pb))
                if b == 0 and h + 1 < H:
                    stage_bias(h + 1)
                cur = psU.tile([128, 1024], FP32, tag="u")
                for kt in range(8):
                    ps = psS.tile([128, 1024], FP32, tag="ps")
                    for qc in range(2):
                        nc.tensor.matmul(
                            ps[:, qc * 512 : (qc + 1) * 512],
                            kT[b][hp : hp + 64, hc + kt * 128 : hc + (kt + 1) * 128],
                            qT[b][hp : hp + 64, hc + qc * 512 : hc + (qc + 1) * 512],
                            start=True,
                            stop=True,
                        )
                    if prev is not None:
                        emit_attnv_slot(prev[0], prev[1], prev[2], kt)
                    es = espool.tile([128, 1024], FP16, tag="es")
                    nc.scalar.activation(es[:], ps[:], Exp)
                    ex = epool.tile([128, 1024], FP16, tag="ex")
                    nc.vector.tensor_tensor(
                        ex[:],
                        es[:],
                        bias_tiles[h][kt // 4][:, (kt % 4) * L : (kt % 4 + 1) * L],
                        Mult,
                    )
                    ex_tiles[(b, kt)] = ex
                    bg_step(2 if w == 0 else 1)
                if prev is not None:
                    emit_norm(prev[0], prev[1], prev[2])
                    if prev[0] == H - 1:
                        bg.append(phase_c_steps(prev[1]))
                prev = (h, b, cur)

            # drain: attn@v for the last window (h=H-1, b=1)
            for kt in range(8):
                emit_attnv_slot(prev[0], prev[1], prev[2], kt)
                bg_step(2)
            emit_norm(prev[0], prev[1], prev[2])
            bg.append(phase_c_steps(prev[1]))
            while bg:
                bg_step()

    nc.compile()
    return nc


_NC = None


def _get_nc():
    global _NC
    if _NC is None:
        _NC = _build()
    return _NC


def _prep_in_maps(inputs):
    bf16 = ml_dtypes.bfloat16
    xq = np.asarray(inputs["inputs_q"], dtype=np.float32)
    xkv = np.asarray(inputs["inputs_kv"], dtype=np.float32)
    Wq = (np.asarray(inputs["Wq"], dtype=np.float32) * 0.125).astype(bf16)
    bq = np.asarray(inputs["bq"], dtype=np.float32) * 0.125
    Wk = np.asarray(inputs["Wk"], dtype=np.float32).astype(bf16)
    bk = np.asarray(inputs["bk"], dtype=np.float32)
    Wv = np.asarray(inputs["Wv"], dtype=np.float32).astype(bf16)
    Wo = np.asarray(inputs["Wo"], dtype=np.float32).astype(bf16)
    bv = np.asarray(inputs["bv"], dtype=np.float32)
    bo = np.asarray(inputs["bo"], dtype=np.float32)
    bvb = np.tile(bv[None, :], (128, 2)).astype(bf16)
    bob = np.tile(bo[None, :], (128, 2)).astype(bf16)
    toe = np.asarray(inputs["toeplitz"], dtype=np.float32)

    xqT = np.ascontiguousarray(xq.transpose(0, 2, 1)).astype(bf16)  # [B, F, L]
    xkvT = np.ascontiguousarray(xkv.transpose(0, 2, 1)).astype(bf16)

    coords = np.arange(L)
    xi, yi = coords // NY, coords % NY
    dx = xi[:, None] - xi[None, :] + NX
    dy = yi[:, None] - yi[None, :] + NY
    idx = dx * (2 * NY) + dy  # [L(q), L(k)]
    bias = toe[:, idx]  # [H, L(q), L(k)]
    biasT = np.exp(np.ascontiguousarray(bias.transpose(0, 2, 1))).astype(np.float16)

    in_maps = []
    for i in range(NCORES):
        sl = slice(i * BPC, (i + 1) * BPC)
        in_maps.append(
            {
                "xqT": np.ascontiguousarray(xqT[sl]),
                "xkvT": np.ascontiguousarray(xkvT[sl]),
                "Wq": Wq, "Wk": Wk, "Wv": Wv, "Wo": Wo,
                "bq": bq, "bk": bk, "bvb": bvb, "bob": bob,
                "biasT": biasT,
            }
        )
    return in_maps


def _run(inputs, trace=False):
    from concourse.bass_interp import get_hw_module

    nc = _get_nc()
    in_maps = _prep_in_maps(inputs)
    old_m = nc.m
    nc.m = get_hw_module(nc.m)
    try:
        res = run_bass_kernel_spmd(
            nc, in_maps, core_ids=list(range(NCORES)), trace=trace
        )
    finally:
        nc.m = old_m
    out = np.concatenate(
        [np.asarray(r["out"], dtype=np.float32) for r in res.results], axis=0
    )  # [B, L, F]
    return out.reshape(B, L, H, D), res


def kernel(**inputs) -> np.ndarray:
    out, _ = _run(inputs, trace=False)
    return out
